# revision 6
# baseline (speedup 1.0000x reference)
"""CBOW negative-sampling loss kernel for 8 Trainium2 NeuronCores.

The reference computes one-hot @ table matmuls (embedding lookups in
disguise) followed by a tiny log-sigmoid loss.  Device-side algorithm
(v2: single fused DVE pass per byte, no DRAM scratch, no host iota):

Streaming extraction (DMA-bound, ~305us/core at the 360 GB/s limit):
  One-hot rows stream as 5 SBUF tiles of [128, *]:
    T0: the 32 vo rows split in 4 partition-quarters [128, 12500]
    T1..T4: the 192 vi + 320 neg rows as 4x [128, 50000]
  Chunks are [128, 6250]; T4's last 5 chunks are split into [128, 1250]
  pieces (tapering 625/313/312) so DVE tracks the DMA to the last byte.
  ONE fused DVE scalar_tensor_tensor per chunk multiplies by a
  Pool-generated iota tile (value MARK + j, MARK = 65536) and
  row-reduces into vals[:, col]; each one-hot row has <= one 1, so
  vals[p, col] = MARK + in-chunk-offset, exactly in fp32.
  Per tile the hit chunk c is recovered BIT-WISE (no HW divide): for
  each bit b, a weighted column reduce with weights bit_b(chunk(col))
  gives S2_b; bit_b = S2_b >= MARK.  Then
    off = max(sum_b 6250*2^b*bit_b + (S1 - MARK), 0),   cnt = S1 >= MARK
  all on DVE (no DMA deps, so the in-order DVE queue never stalls).
  For T0 a [128]->[32] one-hot fold matmul sums the 4 quarter
  contributions cnt*(MARK + 12500 q) + off into the global vo index.

Gathers + dots (overlapped with streaming of later tiles):
  V rows for vo are gathered once [32, 300] and replicated to each
  tile's partition order by a one-hot [32->128] matmul into PSUM (ACT
  copies PSUM->SBUF).  U rows are gathered per tile with single-offset
  indirect DMA driven straight from the extracted SBUF indices.  The
  per-row dot d = U_row . V_vo_row runs as Pool multiply + ACT
  accumulate mid-stream (never on the DVE queue, which would stall
  behind the gather), and as one fused DVE op for the final tile.

Host: batch-shard across 8 cores, log-sigmoid loss terms + mean of the
256 per-batch terms (same split as the v1 baseline).

Engine/ISA notes (hardware-verified): tensor_tensor_reduce (bass_isa
extended ISA) compiles but faults at runtime here - scalar_tensor_tensor
(core BIR, is_scalar_tensor_tensor=True) is the fused multiply+reduce
that actually runs.  Pool supports tensor_tensor {mult,add,sub} and
tensor_scalar {mult,sub,max,is_ge} only (no divide anywhere).
tensor_scalar with accum_out faults at runtime.
"""
import numpy as np

import concourse.bass as bass
import concourse.mybir as mybir
from concourse.tile import TileContext
from concourse.bass_utils import run_bass_kernel_spmd

VOC = 50000
EMB = 300
B = 256
CTX = 6
K = 10
NCORES = 8
BPC = B // NCORES                    # 32 batch rows per core
NV = BPC * CTX                       # 192 vi rows per core
NN = BPC * K                         # 320 neg rows per core
CH = 6250                            # free-dim chunk width
NCH = VOC // CH                      # 8 chunks per full 50000 row
QW = VOC // 4                        # 12500 per vo partition-quarter
MARK = 65536.0                       # presence marker (> max idx, power of 2)

F32 = mybir.dt.float32
I32 = mybir.dt.int32


def _split_multi_waits(nc):
    """This env's walrus accepts only ONE sync wait per instruction.
    Hoist extra waits into single-wait NoOps right before the owner."""
    cnt = 0
    for fn in nc.m.functions:
        for blk in fn.blocks:
            insts = list(blk.instructions)
            if not any(
                i.sync_info and i.sync_info.on_wait and len(i.sync_info.on_wait) > 1
                for i in insts
            ):
                continue
            new = []
            for inst in insts:
                si = inst.sync_info
                if si and si.on_wait and len(si.on_wait) > 1:
                    waits = list(si.on_wait)
                    for w in waits[:-1]:
                        cnt += 1
                        nop = mybir.InstNoOp(
                            name=f"mwsplit-{cnt}", engine=inst.engine, ins=[], outs=[]
                        )
                        nop.sync_info = mybir.SyncInfo(on_wait=[w], on_update=[])
                        new.append(nop)
                    inst.sync_info = mybir.SyncInfo(
                        on_wait=[waits[-1]], on_update=list(si.on_update or [])
                    )
                new.append(inst)
            blk.instructions = new
    return cnt


def _build():
    nc = bass.Bass(enable_partition_id=False)

    vo = nc.declare_dram_parameter("vo", [BPC, VOC], F32, isOutput=False)
    vi = nc.declare_dram_parameter("vi", [NV, VOC], F32, isOutput=False)
    ng = nc.declare_dram_parameter("ng", [NN, VOC], F32, isOutput=False)
    V = nc.declare_dram_parameter("V", [VOC, EMB], F32, isOutput=False)
    U = nc.declare_dram_parameter("U", [VOC, EMB], F32, isOutput=False)
    # merged consts: col 0 qbaseM | 1:3 wc2-bit0 | 3:27 wc8 bits | 27:117
    # wc30 bits | 117:149 foldq | 149:152 bit weights CH*2^b
    cc = nc.declare_dram_parameter("cc", [128, 152], F32, isOutput=False)
    reps = nc.declare_dram_parameter("reps", [32, 4 * 128], F32, isOutput=False)
    d_out = nc.declare_dram_parameter("dout", [128, 4], F32, isOutput=True)
    c_out = nc.declare_dram_parameter("cout", [128, 4], F32, isOutput=True)

    vo_q = vo.rearrange("r (q f) -> (r q) f", q=4)     # [128, 12500]
    # big tiles: list of (dram slice, partition range) DMAs per tile
    big = [
        [(vi[0:128, :], 0, 128)],
        [(vi[128:NV, :], 0, 64), (ng[0:64, :], 64, 128)],
        [(ng[64:192, :], 0, 128)],
        [(ng[192:NN, :], 0, 128)],
    ]

    AX = mybir.AxisListType.X
    OP = mybir.AluOpType
    ACTF = mybir.ActivationFunctionType

    with TileContext(nc) as tc:
        with (
            tc.tile_pool(name="const", bufs=1) as cpool,
            tc.tile_pool(name="data", bufs=3) as dpool,
            tc.tile_pool(name="pieces", bufs=12) as qpool,
            tc.tile_pool(name="pieces2", bufs=2) as q2pool,
            tc.tile_pool(name="small", bufs=2) as spool,
            tc.tile_pool(name="keep", bufs=1) as kpool,
            tc.tile_pool(name="gath", bufs=2) as gpool,
            tc.tile_pool(name="psum", bufs=2, space="PSUM") as psum_pool,
        ):
            # on-device iota (value MARK + j, exact in fp32); consts go on
            # the ACT DMA queue so SP can start the big streaming DMAs at
            # once (keeps the serial DMA device fed from t=0)
            iota_t = cpool.tile([128, CH], F32, tag="iota")
            nc.gpsimd.iota(
                out=iota_t[:], pattern=[[1, CH]], base=int(MARK),
                channel_multiplier=0, allow_small_or_imprecise_dtypes=True,
            )
            dall = kpool.tile([128, 4], F32, tag="dall")
            call = kpool.tile([128, 4], F32, tag="call")

            def stream_tile(srcs, nch, vals, split_from=None):
                """DMA [128, CH] chunks + fused multiply-reduce each into
                vals[:, col].  Chunks >= split_from are 5 piecewise
                [128, 1250] DMAs+reduces (one vals column each, weight
                handled by the wc table) so DVE tracks the DMA closely and
                the kernel-tail dependency chain stays short."""
                if split_from is None:
                    split_from = nch
                col = 0
                for c in range(split_from):
                    chunk = dpool.tile([128, CH], F32, tag="chunk")
                    for src, p0, p1 in srcs:
                        nc.sync.dma_start(
                            out=chunk[p0:p1, :], in_=src[:, c * CH:(c + 1) * CH]
                        )
                    nc.vector.scalar_tensor_tensor(
                        out=chunk[:], in0=chunk[:], scalar=1.0, in1=iota_t[:],
                        op0=OP.mult, op1=OP.mult,
                        accum_out=vals[:, col:col + 1],
                    )
                    col += 1
                W = CH // 5
                subs = []
                for c in range(split_from, nch):
                    for h in range(5):
                        if c == nch - 1 and h == 4:
                            subs.extend([(c, h * W, 625), (c, h * W + 625, 313),
                                         (c, h * W + 938, 312)])
                        else:
                            subs.append((c, h * W, W))
                for c, j0, w in subs:
                    pp = qpool if w == W else q2pool
                    piece = pp.tile([128, w], F32, tag=f"piece{w}")
                    for src, p0, p1 in srcs:
                        nc.sync.dma_start(
                            out=piece[p0:p1, :],
                            in_=src[:, c * CH + j0:c * CH + j0 + w],
                        )
                    nc.vector.scalar_tensor_tensor(
                        out=piece[:], in0=piece[:], scalar=1.0,
                        in1=iota_t[:, j0:j0 + w],
                        op0=OP.mult, op1=OP.mult,
                        accum_out=vals[:, col:col + 1],
                    )
                    col += 1

            def extract(vals, wcbits, ncol, cnt_out, off_out, tag):
                """cnt = (row had a 1); off = CH*c_hit + j_hit (0 if none).
                The hit-chunk index c is recovered bit-by-bit: one weighted
                column reduce per bit of c (weight = that bit of each
                column's chunk index), then is_ge(MARK) -> bit, scaled by
                CH*2^b and summed.  No division needed anywhere.  Heavy
                reduces on DVE (no DMA deps); scalar chain on Pool."""
                S1 = spool.tile([128, 1], F32, tag=f"S1{tag}")
                nc.vector.tensor_reduce(out=S1[:], in_=vals[:], axis=AX,
                                        op=OP.add)
                nbits = len(wcbits)
                bbs = spool.tile([128, nbits], F32, tag=f"bbs{tag}")
                for b, wc in enumerate(wcbits):
                    S2 = spool.tile([128, 1], F32, tag=f"S2{tag}{b}")
                    junk = spool.tile([128, ncol], F32, tag=f"jk{ncol}")
                    nc.vector.scalar_tensor_tensor(
                        out=junk[:], in0=vals[:], scalar=1.0, in1=wc,
                        op0=OP.mult, op1=OP.mult, accum_out=S2[:],
                    )
                    nc.vector.tensor_scalar(
                        out=bbs[:, b:b + 1], in0=S2[:], scalar1=MARK,
                        scalar2=None, op0=OP.is_ge,
                    )
                nc.vector.tensor_scalar(
                    out=cnt_out, in0=S1[:], scalar1=MARK, scalar2=None, op0=OP.is_ge
                )
                # acc = sum_b bit_b * CH*2^b in ONE fused weighted reduce
                junkb = spool.tile([128, nbits], F32, tag=f"jb{tag}")
                acc = spool.tile([128, 1], F32, tag=f"acc{tag}")
                nc.vector.scalar_tensor_tensor(
                    out=junkb[:], in0=bbs[:], scalar=1.0,
                    in1=wpow_t[:, :nbits],
                    op0=OP.mult, op1=OP.mult, accum_out=acc[:],
                )
                # off = max(acc - MARK + S1, 0): a no-hit row gives acc=0,
                # S1=0 -> clamps to 0; keeps cnt off this path.  The clamp
                # writes off_out directly (an i32 tile for the gather path).
                aj = spool.tile([128, 1], F32, tag=f"aj{tag}")
                nc.vector.scalar_tensor_tensor(
                    out=aj[:], in0=acc[:], scalar=-MARK, in1=S1[:],
                    op0=OP.add, op1=OP.add,
                )
                nc.vector.tensor_scalar(
                    out=off_out, in0=aj[:], scalar1=0.0, scalar2=None, op0=OP.max
                )

            # ---------------- T0: vo quarters ----------------
            vals0 = spool.tile([128, 2], F32, tag="vals0")
            stream_tile([(vo_q, 0, 128)], 2, vals0)

            # small constants: two merged DMAs on the ACT queue (after the
            # first big chunks, one HWDGE generation each)
            cc_t = cpool.tile([128, 152], F32, tag="cc")
            nc.scalar.dma_start(out=cc_t[:], in_=cc[:])
            reps_all = cpool.tile([32, 4 * 128], F32, tag="reps")
            nc.scalar.dma_start(out=reps_all[:], in_=reps[:])
            qbaseM_t = cc_t[:, 0:1]
            wc2_bits = [cc_t[:, 1:3]]
            wc8_bits = [cc_t[:, 3 + 8 * b:11 + 8 * b] for b in range(3)]
            wc28_bits = [cc_t[:, 27 + 30 * b:57 + 30 * b] for b in range(3)]
            foldq_t = cc_t[:, 117:149]
            wpow_t = cc_t[:, 149:152]
            reps_t = [reps_all[:, 128 * t:128 * (t + 1)] for t in range(4)]

            cnt0 = spool.tile([128, 1], F32, tag="cnt0")
            off0 = spool.tile([128, 1], F32, tag="off0")
            extract(vals0, wc2_bits, 2, cnt0[:], off0[:], "t0")
            # X = cnt*(MARK + 12500 q) + off ; fold quarters -> [32, 1]
            xq = spool.tile([128, 1], F32, tag="xq")
            nc.vector.tensor_tensor(out=xq[:], in0=cnt0[:], in1=qbaseM_t, op=OP.mult)
            x2 = spool.tile([128, 1], F32, tag="x2")
            nc.vector.tensor_tensor(out=x2[:], in0=xq[:], in1=off0[:], op=OP.add)
            pfold = psum_pool.tile([32, 1], F32, tag="p32")
            nc.tensor.matmul(out=pfold[:], lhsT=foldq_t, rhs=x2[:],
                             start=True, stop=True)
            ofs_v = spool.tile([32, 1], I32, tag="ofsv")
            nc.vector.tensor_scalar(
                out=ofs_v[:], in0=pfold[:], scalar1=MARK, scalar2=None,
                op0=OP.subtract,
            )
            voV = cpool.tile([32, EMB], F32, tag="voV")
            nc.gpsimd.indirect_dma_start(
                out=voV[:], out_offset=None, in_=V[:],
                in_offset=bass.IndirectOffsetOnAxis(ap=ofs_v[:], axis=0),
            )

            # ---------------- T1..T4: vi + neg rows ----------------
            # The per-tile dot runs as Pool multiply + ACT accumulate —
            # neither touches the DVE streaming queue, so a gather that
            # lands late can only block Pool (whose next deadline is a
            # full tile away).
            for t in range(4):
                last = t == 3
                ncol = 3 + 27 if last else NCH
                vals = spool.tile([128, ncol], F32, tag=f"vals{ncol}")
                stream_tile(big[t], NCH, vals, split_from=3 if last else None)
                ofs_u = spool.tile([128, 1], I32, tag="ofsu")
                extract(vals, wc28_bits if last else wc8_bits, ncol,
                        call[:, t:t + 1], ofs_u[:], "tb")
                rowU = gpool.tile([128, EMB], F32, tag="rowU")
                nc.gpsimd.indirect_dma_start(
                    out=rowU[:], out_offset=None, in_=U[:],
                    in_offset=bass.IndirectOffsetOnAxis(ap=ofs_u[:], axis=0),
                )
                pB = psum_pool.tile([128, EMB], F32, tag="pB")
                nc.tensor.matmul(out=pB[:], lhsT=reps_t[t], rhs=voV[:],
                                 start=True, stop=True)
                voB = gpool.tile([128, EMB], F32, tag="voB")
                nc.scalar.activation(out=voB[:], in_=pB[:], func=ACTF.Copy)
                prodB = gpool.tile([128, EMB], F32, tag="prodB")
                if last:
                    # tail: one fused DVE op (DVE is idle by now)
                    nc.vector.scalar_tensor_tensor(
                        out=prodB[:], in0=rowU[:], scalar=1.0, in1=voB[:],
                        op0=OP.mult, op1=OP.mult, accum_out=dall[:, t:t + 1],
                    )
                else:
                    # mid-stream: keep the gather-dependent dot off DVE
                    nc.gpsimd.tensor_tensor(
                        out=prodB[:], in0=rowU[:], in1=voB[:], op=OP.mult
                    )
                    nc.scalar.activation(
                        out=prodB[:], in_=prodB[:], func=ACTF.Copy,
                        accum_out=dall[:, t:t + 1],
                    )

            # c_out rides the idle SP queue (off the critical tail); d_out
            # on ACT (one DVE->ACT hop after the fused T4 dot)
            nc.sync.dma_start(out=c_out[:], in_=call[:])
            nc.scalar.dma_start(out=d_out[:], in_=dall[:])

    _split_multi_waits(nc)
    mybir.codegen_inst_isa_subclasses(nc)
    return nc


def _consts():
    p = np.arange(128)
    qbaseM = (MARK + (p % 4) * QW).astype(np.float32).reshape(128, 1)
    wc2 = np.tile(np.arange(2, dtype=np.float32), (128, 1))
    wc8 = np.tile(np.arange(8, dtype=np.float32), (128, 1))
    c8 = np.arange(8)
    c28 = np.concatenate([np.arange(3), np.repeat(np.arange(3, 8), 5), [7, 7]])
    wc8b = np.concatenate(
        [np.tile(((c8 >> b) & 1).astype(np.float32), (128, 1)) for b in range(3)],
        axis=1)
    wc28b = np.concatenate(
        [np.tile(((c28 >> b) & 1).astype(np.float32), (128, 1)) for b in range(3)],
        axis=1)
    foldq = np.zeros((128, 32), np.float32)
    foldq[p, p // 4] = 1.0
    wpow = np.tile(np.array([6250.0, 12500.0, 25000.0],
                            dtype=np.float32), (128, 1))
    cc = np.concatenate([qbaseM, wc2, wc8b, wc28b, foldq, wpow], axis=1)
    # reps[t, b, p] = 1 iff partition p of tile t holds a row of batch b
    bmap = np.empty((4, 128), np.int64)
    bmap[0] = p // CTX                                   # vi rows 0..127
    bmap[1, :64] = (128 + p[:64]) // CTX                 # vi rows 128..191
    bmap[1, 64:] = (p[64:] - 64) // K                    # ng rows 0..63
    bmap[2] = (64 + p) // K                              # ng rows 64..191
    bmap[3] = (192 + p) // K                             # ng rows 192..319
    reps = np.zeros((4, 32, 128), np.float32)
    for t in range(4):
        reps[t, bmap[t], p] = 1.0
    reps = reps.transpose(1, 0, 2).reshape(32, 4 * 128)
    return cc, reps


_CACHE = {}


def kernel(vo, vi, neg_samples, V, U):
    if "nc" not in _CACHE:
        _CACHE["nc"] = _build()
        _CACHE["consts"] = _consts()
    nc = _CACHE["nc"]
    cc, reps = _CACHE["consts"]

    vo = np.ascontiguousarray(vo, dtype=np.float32)
    vi = np.ascontiguousarray(vi, dtype=np.float32)
    neg = np.ascontiguousarray(neg_samples, dtype=np.float32)
    V = np.ascontiguousarray(V, dtype=np.float32)
    U = np.ascontiguousarray(U, dtype=np.float32)

    in_maps = []
    for c in range(NCORES):
        sl = slice(c * BPC, (c + 1) * BPC)
        in_maps.append({
            "vo": vo[sl],
            "vi": vi[sl].reshape(NV, VOC),
            "ng": neg[sl].reshape(NN, VOC),
            "V": V,
            "U": U,
            "cc": cc, "reps": reps,
        })

    res = run_bass_kernel_spmd(nc, in_maps, list(range(NCORES)))
    obs = []
    for r in res.results:
        d = r["dout"]                                  # [128, 4]
        cc = r["cout"]                                 # [128, 4]
        d_vi = np.concatenate([d[:, 0], d[:64, 1]]).reshape(BPC, CTX)
        c_vi = np.concatenate([cc[:, 0], cc[:64, 1]]).reshape(BPC, CTX)
        d_ng = np.concatenate([d[64:, 1], d[:, 2], d[:, 3]]).reshape(BPC, K)
        lp = (d_vi * c_vi).sum(axis=1)
        ms = c_vi.sum(axis=1)
        x = lp / ms
        left = -np.log1p(np.exp(-x))
        right = (-np.log1p(np.exp(d_ng))).sum(axis=1)
        obs.append(-(left + right))
    ob = np.concatenate(obs)
    return np.float32(ob.mean(dtype=np.float64))


# revision 7
# speedup vs baseline: 1.0015x; 1.0015x over previous
"""CBOW negative-sampling loss kernel for 8 Trainium2 NeuronCores.

The reference computes one-hot @ table matmuls (embedding lookups in
disguise) followed by a tiny log-sigmoid loss.  Device-side algorithm
(v2: single fused DVE pass per byte, no DRAM scratch, no host iota):

Streaming extraction (DMA-bound, ~305us/core at the 360 GB/s limit):
  One-hot rows stream as 5 SBUF tiles of [128, *]:
    T0: the 32 vo rows split in 4 partition-quarters [128, 12500]
    T1..T4: the 192 vi + 320 neg rows as 4x [128, 50000]
  Chunks are [128, 6250]; T4's last 5 chunks are split into [128, 1250]
  pieces (the final chunk into 10x625) so the DVE reduce stays caught
  up to the very last streamed byte.
  ONE fused DVE scalar_tensor_tensor per chunk multiplies by a
  Pool-generated iota tile (value MARK + j, MARK = 65536) and
  row-reduces into vals[:, col]; each one-hot row has <= one 1, so
  vals[p, col] = MARK + in-chunk-offset, exactly in fp32.
  Per tile the hit chunk c is recovered BIT-WISE (no HW divide): for
  each bit b, a weighted column reduce with weights bit_b(chunk(col))
  gives S2_b; bit_b = S2_b >= MARK.  Then
    off = max(sum_b 6250*2^b*bit_b + (S1 - MARK), 0),   cnt = S1 >= MARK
  all on DVE (no DMA deps, so the in-order DVE queue never stalls).
  For T0 a [128]->[32] one-hot fold matmul sums the 4 quarter
  contributions cnt*(MARK + 12500 q) + off into the global vo index.

Gathers + dots (overlapped with streaming of later tiles):
  V rows for vo are gathered once [32, 300] and replicated to each
  tile's partition order by a one-hot [32->128] matmul into PSUM (ACT
  copies PSUM->SBUF).  U rows are gathered per tile with single-offset
  indirect DMA driven straight from the extracted SBUF indices.  The
  per-row dot d = U_row . V_vo_row runs as Pool multiply + ACT
  accumulate mid-stream (never on the DVE queue, which would stall
  behind the gather), and as one fused DVE op for the final tile.

Host: batch-shard across 8 cores, log-sigmoid loss terms + mean of the
256 per-batch terms (same split as the v1 baseline).

Engine/ISA notes (hardware-verified): tensor_tensor_reduce (bass_isa
extended ISA) compiles but faults at runtime here - scalar_tensor_tensor
(core BIR, is_scalar_tensor_tensor=True) is the fused multiply+reduce
that actually runs.  Pool supports tensor_tensor {mult,add,sub} and
tensor_scalar {mult,sub,max,is_ge} only (no divide anywhere).
tensor_scalar with accum_out faults at runtime.
"""
import numpy as np

import concourse.bass as bass
import concourse.mybir as mybir
from concourse.tile import TileContext
from concourse.bass_utils import run_bass_kernel_spmd

VOC = 50000
EMB = 300
B = 256
CTX = 6
K = 10
NCORES = 8
BPC = B // NCORES                    # 32 batch rows per core
NV = BPC * CTX                       # 192 vi rows per core
NN = BPC * K                         # 320 neg rows per core
CH = 6250                            # free-dim chunk width
NCH = VOC // CH                      # 8 chunks per full 50000 row
QW = VOC // 4                        # 12500 per vo partition-quarter
MARK = 65536.0                       # presence marker (> max idx, power of 2)

F32 = mybir.dt.float32
I32 = mybir.dt.int32


def _split_multi_waits(nc):
    """This env's walrus accepts only ONE sync wait per instruction.
    Hoist extra waits into single-wait NoOps right before the owner."""
    cnt = 0
    for fn in nc.m.functions:
        for blk in fn.blocks:
            insts = list(blk.instructions)
            if not any(
                i.sync_info and i.sync_info.on_wait and len(i.sync_info.on_wait) > 1
                for i in insts
            ):
                continue
            new = []
            for inst in insts:
                si = inst.sync_info
                if si and si.on_wait and len(si.on_wait) > 1:
                    waits = list(si.on_wait)
                    for w in waits[:-1]:
                        cnt += 1
                        nop = mybir.InstNoOp(
                            name=f"mwsplit-{cnt}", engine=inst.engine, ins=[], outs=[]
                        )
                        nop.sync_info = mybir.SyncInfo(on_wait=[w], on_update=[])
                        new.append(nop)
                    inst.sync_info = mybir.SyncInfo(
                        on_wait=[waits[-1]], on_update=list(si.on_update or [])
                    )
                new.append(inst)
            blk.instructions = new
    return cnt


def _build():
    nc = bass.Bass(enable_partition_id=False)

    vo = nc.declare_dram_parameter("vo", [BPC, VOC], F32, isOutput=False)
    vi = nc.declare_dram_parameter("vi", [NV, VOC], F32, isOutput=False)
    ng = nc.declare_dram_parameter("ng", [NN, VOC], F32, isOutput=False)
    V = nc.declare_dram_parameter("V", [VOC, EMB], F32, isOutput=False)
    U = nc.declare_dram_parameter("U", [VOC, EMB], F32, isOutput=False)
    # merged consts: col 0 qbaseM | 1:3 wc2-bit0 | 3:27 wc8 bits | 27:126
    # wc33 bits | 126:158 foldq | 158:161 bit weights CH*2^b
    cc = nc.declare_dram_parameter("cc", [128, 161], F32, isOutput=False)
    reps = nc.declare_dram_parameter("reps", [32, 4 * 128], F32, isOutput=False)
    d_out = nc.declare_dram_parameter("dout", [128, 4], F32, isOutput=True)
    c_out = nc.declare_dram_parameter("cout", [128, 4], F32, isOutput=True)

    vo_q = vo.rearrange("r (q f) -> (r q) f", q=4)     # [128, 12500]
    # big tiles: list of (dram slice, partition range) DMAs per tile
    big = [
        [(vi[0:128, :], 0, 128)],
        [(vi[128:NV, :], 0, 64), (ng[0:64, :], 64, 128)],
        [(ng[64:192, :], 0, 128)],
        [(ng[192:NN, :], 0, 128)],
    ]

    AX = mybir.AxisListType.X
    OP = mybir.AluOpType
    ACTF = mybir.ActivationFunctionType

    with TileContext(nc) as tc:
        with (
            tc.tile_pool(name="const", bufs=1) as cpool,
            tc.tile_pool(name="data", bufs=3) as dpool,
            tc.tile_pool(name="pieces", bufs=10) as qpool,
            tc.tile_pool(name="small", bufs=2) as spool,
            tc.tile_pool(name="keep", bufs=1) as kpool,
            tc.tile_pool(name="gath", bufs=2) as gpool,
            tc.tile_pool(name="psum", bufs=2, space="PSUM") as psum_pool,
        ):
            # on-device iota (value MARK + j, exact in fp32); consts go on
            # the ACT DMA queue so SP can start the big streaming DMAs at
            # once (keeps the serial DMA device fed from t=0)
            iota_t = cpool.tile([128, CH], F32, tag="iota")
            nc.gpsimd.iota(
                out=iota_t[:], pattern=[[1, CH]], base=int(MARK),
                channel_multiplier=0, allow_small_or_imprecise_dtypes=True,
            )
            dall = kpool.tile([128, 4], F32, tag="dall")
            call = kpool.tile([128, 4], F32, tag="call")

            def stream_tile(srcs, nch, vals, split_from=None):
                """DMA [128, CH] chunks + fused multiply-reduce each into
                vals[:, col].  Chunks >= split_from are 5 piecewise
                [128, 1250] DMAs+reduces (one vals column each, weight
                handled by the wc table) so DVE tracks the DMA closely and
                the kernel-tail dependency chain stays short."""
                if split_from is None:
                    split_from = nch
                col = 0
                for c in range(split_from):
                    chunk = dpool.tile([128, CH], F32, tag="chunk")
                    for src, p0, p1 in srcs:
                        nc.sync.dma_start(
                            out=chunk[p0:p1, :], in_=src[:, c * CH:(c + 1) * CH]
                        )
                    nc.vector.scalar_tensor_tensor(
                        out=chunk[:], in0=chunk[:], scalar=1.0, in1=iota_t[:],
                        op0=OP.mult, op1=OP.mult,
                        accum_out=vals[:, col:col + 1],
                    )
                    col += 1
                W = CH // 5
                subs = []
                for c in range(split_from, nch):
                    if c == nch - 1:
                        # final chunk: uniform 625-wide pieces; DVE is
                        # faster per piece (817 ns vs 889 ns DMA) so it
                        # stays caught up to the very last byte
                        subs.extend([(c, h * 625, 625) for h in range(10)])
                    else:
                        subs.extend([(c, h * W, W) for h in range(5)])
                for c, j0, w in subs:
                    piece = qpool.tile([128, w], F32, tag=f"piece{w}")
                    for src, p0, p1 in srcs:
                        nc.sync.dma_start(
                            out=piece[p0:p1, :],
                            in_=src[:, c * CH + j0:c * CH + j0 + w],
                        )
                    nc.vector.scalar_tensor_tensor(
                        out=piece[:], in0=piece[:], scalar=1.0,
                        in1=iota_t[:, j0:j0 + w],
                        op0=OP.mult, op1=OP.mult,
                        accum_out=vals[:, col:col + 1],
                    )
                    col += 1

            def extract(vals, wcbits, ncol, cnt_out, off_out, tag):
                """cnt = (row had a 1); off = CH*c_hit + j_hit (0 if none).
                The hit-chunk index c is recovered bit-by-bit: one weighted
                column reduce per bit of c (weight = that bit of each
                column's chunk index), then is_ge(MARK) -> bit, scaled by
                CH*2^b and summed.  No division needed anywhere.  Heavy
                reduces on DVE (no DMA deps); scalar chain on Pool."""
                S1 = spool.tile([128, 1], F32, tag=f"S1{tag}")
                nc.vector.tensor_reduce(out=S1[:], in_=vals[:], axis=AX,
                                        op=OP.add)
                nbits = len(wcbits)
                bbs = spool.tile([128, nbits], F32, tag=f"bbs{tag}")
                for b, wc in enumerate(wcbits):
                    S2 = spool.tile([128, 1], F32, tag=f"S2{tag}{b}")
                    junk = spool.tile([128, ncol], F32, tag=f"jk{ncol}")
                    nc.vector.scalar_tensor_tensor(
                        out=junk[:], in0=vals[:], scalar=1.0, in1=wc,
                        op0=OP.mult, op1=OP.mult, accum_out=S2[:],
                    )
                    nc.vector.tensor_scalar(
                        out=bbs[:, b:b + 1], in0=S2[:], scalar1=MARK,
                        scalar2=None, op0=OP.is_ge,
                    )
                nc.vector.tensor_scalar(
                    out=cnt_out, in0=S1[:], scalar1=MARK, scalar2=None, op0=OP.is_ge
                )
                # acc = sum_b bit_b * CH*2^b in ONE fused weighted reduce
                junkb = spool.tile([128, nbits], F32, tag=f"jb{tag}")
                acc = spool.tile([128, 1], F32, tag=f"acc{tag}")
                nc.vector.scalar_tensor_tensor(
                    out=junkb[:], in0=bbs[:], scalar=1.0,
                    in1=wpow_t[:, :nbits],
                    op0=OP.mult, op1=OP.mult, accum_out=acc[:],
                )
                # off = max(acc - MARK + S1, 0): a no-hit row gives acc=0,
                # S1=0 -> clamps to 0; keeps cnt off this path.  The clamp
                # writes off_out directly (an i32 tile for the gather path).
                aj = spool.tile([128, 1], F32, tag=f"aj{tag}")
                nc.vector.scalar_tensor_tensor(
                    out=aj[:], in0=acc[:], scalar=-MARK, in1=S1[:],
                    op0=OP.add, op1=OP.add,
                )
                nc.vector.tensor_scalar(
                    out=off_out, in0=aj[:], scalar1=0.0, scalar2=None, op0=OP.max
                )

            # ---------------- T0: vo quarters ----------------
            vals0 = spool.tile([128, 2], F32, tag="vals0")
            stream_tile([(vo_q, 0, 128)], 2, vals0)

            # small constants: two merged DMAs on the ACT queue (after the
            # first big chunks, one HWDGE generation each)
            cc_t = cpool.tile([128, 161], F32, tag="cc")
            nc.scalar.dma_start(out=cc_t[:], in_=cc[:])
            reps_all = cpool.tile([32, 4 * 128], F32, tag="reps")
            nc.scalar.dma_start(out=reps_all[:], in_=reps[:])
            qbaseM_t = cc_t[:, 0:1]
            wc2_bits = [cc_t[:, 1:3]]
            wc8_bits = [cc_t[:, 3 + 8 * b:11 + 8 * b] for b in range(3)]
            wc28_bits = [cc_t[:, 27 + 33 * b:60 + 33 * b] for b in range(3)]
            foldq_t = cc_t[:, 126:158]
            wpow_t = cc_t[:, 158:161]
            reps_t = [reps_all[:, 128 * t:128 * (t + 1)] for t in range(4)]

            cnt0 = spool.tile([128, 1], F32, tag="cnt0")
            off0 = spool.tile([128, 1], F32, tag="off0")
            extract(vals0, wc2_bits, 2, cnt0[:], off0[:], "t0")
            # X = cnt*(MARK + 12500 q) + off ; fold quarters -> [32, 1]
            xq = spool.tile([128, 1], F32, tag="xq")
            nc.vector.tensor_tensor(out=xq[:], in0=cnt0[:], in1=qbaseM_t, op=OP.mult)
            x2 = spool.tile([128, 1], F32, tag="x2")
            nc.vector.tensor_tensor(out=x2[:], in0=xq[:], in1=off0[:], op=OP.add)
            pfold = psum_pool.tile([32, 1], F32, tag="p32")
            nc.tensor.matmul(out=pfold[:], lhsT=foldq_t, rhs=x2[:],
                             start=True, stop=True)
            ofs_v = spool.tile([32, 1], I32, tag="ofsv")
            nc.vector.tensor_scalar(
                out=ofs_v[:], in0=pfold[:], scalar1=MARK, scalar2=None,
                op0=OP.subtract,
            )
            voV = cpool.tile([32, EMB], F32, tag="voV")
            nc.gpsimd.indirect_dma_start(
                out=voV[:], out_offset=None, in_=V[:],
                in_offset=bass.IndirectOffsetOnAxis(ap=ofs_v[:], axis=0),
            )

            # ---------------- T1..T4: vi + neg rows ----------------
            # The per-tile dot runs as Pool multiply + ACT accumulate —
            # neither touches the DVE streaming queue, so a gather that
            # lands late can only block Pool (whose next deadline is a
            # full tile away).
            for t in range(4):
                last = t == 3
                ncol = 3 + 30 if last else NCH
                vals = spool.tile([128, ncol], F32, tag=f"vals{ncol}")
                stream_tile(big[t], NCH, vals, split_from=3 if last else None)
                ofs_u = spool.tile([128, 1], I32, tag="ofsu")
                extract(vals, wc28_bits if last else wc8_bits, ncol,
                        call[:, t:t + 1], ofs_u[:], "tb")
                rowU = gpool.tile([128, EMB], F32, tag="rowU")
                nc.gpsimd.indirect_dma_start(
                    out=rowU[:], out_offset=None, in_=U[:],
                    in_offset=bass.IndirectOffsetOnAxis(ap=ofs_u[:], axis=0),
                )
                pB = psum_pool.tile([128, EMB], F32, tag="pB")
                nc.tensor.matmul(out=pB[:], lhsT=reps_t[t], rhs=voV[:],
                                 start=True, stop=True)
                voB = gpool.tile([128, EMB], F32, tag="voB")
                nc.scalar.activation(out=voB[:], in_=pB[:], func=ACTF.Copy)
                prodB = gpool.tile([128, EMB], F32, tag="prodB")
                if last:
                    # tail: one fused DVE op (DVE is idle by now)
                    nc.vector.scalar_tensor_tensor(
                        out=prodB[:], in0=rowU[:], scalar=1.0, in1=voB[:],
                        op0=OP.mult, op1=OP.mult, accum_out=dall[:, t:t + 1],
                    )
                else:
                    # mid-stream: keep the gather-dependent dot off DVE
                    nc.gpsimd.tensor_tensor(
                        out=prodB[:], in0=rowU[:], in1=voB[:], op=OP.mult
                    )
                    nc.scalar.activation(
                        out=prodB[:], in_=prodB[:], func=ACTF.Copy,
                        accum_out=dall[:, t:t + 1],
                    )

            # c_out rides the idle SP queue (off the critical tail); d_out
            # on ACT (one DVE->ACT hop after the fused T4 dot)
            nc.sync.dma_start(out=c_out[:], in_=call[:])
            nc.scalar.dma_start(out=d_out[:], in_=dall[:])

    _split_multi_waits(nc)
    mybir.codegen_inst_isa_subclasses(nc)
    return nc


def _consts():
    p = np.arange(128)
    qbaseM = (MARK + (p % 4) * QW).astype(np.float32).reshape(128, 1)
    wc2 = np.tile(np.arange(2, dtype=np.float32), (128, 1))
    wc8 = np.tile(np.arange(8, dtype=np.float32), (128, 1))
    c8 = np.arange(8)
    c28 = np.concatenate([np.arange(3), np.repeat(np.arange(3, 7), 5),
                          np.full(10, 7)])
    wc8b = np.concatenate(
        [np.tile(((c8 >> b) & 1).astype(np.float32), (128, 1)) for b in range(3)],
        axis=1)
    wc28b = np.concatenate(
        [np.tile(((c28 >> b) & 1).astype(np.float32), (128, 1)) for b in range(3)],
        axis=1)
    foldq = np.zeros((128, 32), np.float32)
    foldq[p, p // 4] = 1.0
    wpow = np.tile(np.array([6250.0, 12500.0, 25000.0],
                            dtype=np.float32), (128, 1))
    cc = np.concatenate([qbaseM, wc2, wc8b, wc28b, foldq, wpow], axis=1)
    # reps[t, b, p] = 1 iff partition p of tile t holds a row of batch b
    bmap = np.empty((4, 128), np.int64)
    bmap[0] = p // CTX                                   # vi rows 0..127
    bmap[1, :64] = (128 + p[:64]) // CTX                 # vi rows 128..191
    bmap[1, 64:] = (p[64:] - 64) // K                    # ng rows 0..63
    bmap[2] = (64 + p) // K                              # ng rows 64..191
    bmap[3] = (192 + p) // K                             # ng rows 192..319
    reps = np.zeros((4, 32, 128), np.float32)
    for t in range(4):
        reps[t, bmap[t], p] = 1.0
    reps = reps.transpose(1, 0, 2).reshape(32, 4 * 128)
    return cc, reps


_CACHE = {}


def kernel(vo, vi, neg_samples, V, U):
    if "nc" not in _CACHE:
        _CACHE["nc"] = _build()
        _CACHE["consts"] = _consts()
    nc = _CACHE["nc"]
    cc, reps = _CACHE["consts"]

    vo = np.ascontiguousarray(vo, dtype=np.float32)
    vi = np.ascontiguousarray(vi, dtype=np.float32)
    neg = np.ascontiguousarray(neg_samples, dtype=np.float32)
    V = np.ascontiguousarray(V, dtype=np.float32)
    U = np.ascontiguousarray(U, dtype=np.float32)

    in_maps = []
    for c in range(NCORES):
        sl = slice(c * BPC, (c + 1) * BPC)
        in_maps.append({
            "vo": vo[sl],
            "vi": vi[sl].reshape(NV, VOC),
            "ng": neg[sl].reshape(NN, VOC),
            "V": V,
            "U": U,
            "cc": cc, "reps": reps,
        })

    res = run_bass_kernel_spmd(nc, in_maps, list(range(NCORES)))
    obs = []
    for r in res.results:
        d = r["dout"]                                  # [128, 4]
        cc = r["cout"]                                 # [128, 4]
        d_vi = np.concatenate([d[:, 0], d[:64, 1]]).reshape(BPC, CTX)
        c_vi = np.concatenate([cc[:, 0], cc[:64, 1]]).reshape(BPC, CTX)
        d_ng = np.concatenate([d[64:, 1], d[:, 2], d[:, 3]]).reshape(BPC, K)
        lp = (d_vi * c_vi).sum(axis=1)
        ms = c_vi.sum(axis=1)
        x = lp / ms
        left = -np.log1p(np.exp(-x))
        right = (-np.log1p(np.exp(d_ng))).sum(axis=1)
        obs.append(-(left + right))
    ob = np.concatenate(obs)
    return np.float32(ob.mean(dtype=np.float64))


# revision 8
# speedup vs baseline: 1.2783x; 1.2763x over previous
"""CBOW negative-sampling loss kernel for 8 Trainium2 NeuronCores.

The reference computes one-hot @ table matmuls (embedding lookups in
disguise) followed by a tiny log-sigmoid loss.  Device-side algorithm
(v2: single fused DVE pass per byte, no DRAM scratch, no host iota):

Streaming extraction (DVE-bound, ~230us/core):
  The host losslessly recodes the one-hot inputs to uint8 (values are
  exactly {0,1}), so the device streams 27.3 MB/core instead of 109 MB
  - the DMA drops far below the single fused DVE pass, which becomes
  the kernel bottleneck.  One-hot rows stream as 5 SBUF tiles:
    T0: the 32 vo rows split in 4 partition-quarters [128, 12500]
    T1..T4: the 192 vi + 320 neg rows as 4x [128, 50000]
  in [128, 6250] uint8 chunks.
  ONE fused DVE scalar_tensor_tensor per chunk multiplies by a
  Pool-generated iota tile (value MARK + j, MARK = 65536) and
  row-reduces into vals[:, col]; each one-hot row has <= one 1, so
  vals[p, col] = MARK + in-chunk-offset, exactly in fp32.
  Per tile the hit chunk c is recovered BIT-WISE (no HW divide): for
  each bit b, a weighted column reduce with weights bit_b(chunk(col))
  gives S2_b; bit_b = S2_b >= MARK.  Then
    off = max(sum_b 6250*2^b*bit_b + (S1 - MARK), 0),   cnt = S1 >= MARK
  all on DVE (no DMA deps, so the in-order DVE queue never stalls).
  For T0 a [128]->[32] one-hot fold matmul sums the 4 quarter
  contributions cnt*(MARK + 12500 q) + off into the global vo index.

Gathers + dots (overlapped with streaming of later tiles):
  V rows for vo are gathered once [32, 300] and replicated to each
  tile's partition order by a one-hot [32->128] matmul into PSUM (ACT
  copies PSUM->SBUF).  U rows are gathered per tile with single-offset
  indirect DMA driven straight from the extracted SBUF indices.  The
  per-row dot d = U_row . V_vo_row runs as Pool multiply + ACT
  accumulate mid-stream (keeping the bottleneck DVE queue free), and
  as one fused DVE op for the final tile.

Host: batch-shard across 8 cores, log-sigmoid loss terms + mean of the
256 per-batch terms (same split as the v1 baseline).

Engine/ISA notes (hardware-verified): tensor_tensor_reduce (bass_isa
extended ISA) compiles but faults at runtime here - scalar_tensor_tensor
(core BIR, is_scalar_tensor_tensor=True) is the fused multiply+reduce
that actually runs.  Pool supports tensor_tensor {mult,add,sub} and
tensor_scalar {mult,sub,max,is_ge} only (no divide anywhere).
tensor_scalar with accum_out faults at runtime.
"""
import numpy as np

import concourse.bass as bass
import concourse.mybir as mybir
from concourse.tile import TileContext
from concourse.bass_utils import run_bass_kernel_spmd

VOC = 50000
EMB = 300
B = 256
CTX = 6
K = 10
NCORES = 8
BPC = B // NCORES                    # 32 batch rows per core
NV = BPC * CTX                       # 192 vi rows per core
NN = BPC * K                         # 320 neg rows per core
CH = 6250                            # free-dim chunk width
NCH = VOC // CH                      # 8 chunks per full 50000 row
QW = VOC // 4                        # 12500 per vo partition-quarter
MARK = 65536.0                       # presence marker (> max idx, power of 2)

F32 = mybir.dt.float32
I32 = mybir.dt.int32
U8 = mybir.dt.uint8


def _split_multi_waits(nc):
    """This env's walrus accepts only ONE sync wait per instruction.
    Hoist extra waits into single-wait NoOps right before the owner."""
    cnt = 0
    for fn in nc.m.functions:
        for blk in fn.blocks:
            insts = list(blk.instructions)
            if not any(
                i.sync_info and i.sync_info.on_wait and len(i.sync_info.on_wait) > 1
                for i in insts
            ):
                continue
            new = []
            for inst in insts:
                si = inst.sync_info
                if si and si.on_wait and len(si.on_wait) > 1:
                    waits = list(si.on_wait)
                    for w in waits[:-1]:
                        cnt += 1
                        nop = mybir.InstNoOp(
                            name=f"mwsplit-{cnt}", engine=inst.engine, ins=[], outs=[]
                        )
                        nop.sync_info = mybir.SyncInfo(on_wait=[w], on_update=[])
                        new.append(nop)
                    inst.sync_info = mybir.SyncInfo(
                        on_wait=[waits[-1]], on_update=list(si.on_update or [])
                    )
                new.append(inst)
            blk.instructions = new
    return cnt


def _build():
    nc = bass.Bass(enable_partition_id=False)

    vo = nc.declare_dram_parameter("vo", [BPC, VOC], U8, isOutput=False)
    vi = nc.declare_dram_parameter("vi", [NV, VOC], U8, isOutput=False)
    ng = nc.declare_dram_parameter("ng", [NN, VOC], U8, isOutput=False)
    V = nc.declare_dram_parameter("V", [VOC, EMB], F32, isOutput=False)
    U = nc.declare_dram_parameter("U", [VOC, EMB], F32, isOutput=False)
    # merged consts: col 0 qbaseM | 1:3 wc2-bit0 | 3:27 wc8 bits |
    # 27:59 foldq | 59:62 bit weights CH*2^b
    cc = nc.declare_dram_parameter("cc", [128, 62], F32, isOutput=False)
    reps = nc.declare_dram_parameter("reps", [32, 4 * 128], F32, isOutput=False)
    d_out = nc.declare_dram_parameter("dout", [128, 4], F32, isOutput=True)
    c_out = nc.declare_dram_parameter("cout", [128, 4], F32, isOutput=True)

    vo_q = vo.rearrange("r (q f) -> (r q) f", q=4)     # [128, 12500]
    # big tiles: list of (dram slice, partition range) DMAs per tile
    big = [
        [(vi[0:128, :], 0, 128)],
        [(vi[128:NV, :], 0, 64), (ng[0:64, :], 64, 128)],
        [(ng[64:192, :], 0, 128)],
        [(ng[192:NN, :], 0, 128)],
    ]

    AX = mybir.AxisListType.X
    OP = mybir.AluOpType
    ACTF = mybir.ActivationFunctionType

    with TileContext(nc) as tc:
        with (
            tc.tile_pool(name="const", bufs=1) as cpool,
            tc.tile_pool(name="data", bufs=3) as dpool,
            tc.tile_pool(name="prod", bufs=1) as ppool,
            tc.tile_pool(name="small", bufs=2) as spool,
            tc.tile_pool(name="keep", bufs=1) as kpool,
            tc.tile_pool(name="gath", bufs=2) as gpool,
            tc.tile_pool(name="psum", bufs=2, space="PSUM") as psum_pool,
        ):
            # on-device iota (value MARK + j, exact in fp32); consts go on
            # the ACT DMA queue so SP can start the big streaming DMAs at
            # once (keeps the serial DMA device fed from t=0)
            iota_t = cpool.tile([128, CH], F32, tag="iota")
            nc.gpsimd.iota(
                out=iota_t[:], pattern=[[1, CH]], base=int(MARK),
                channel_multiplier=0, allow_small_or_imprecise_dtypes=True,
            )
            dall = kpool.tile([128, 4], F32, tag="dall")
            call = kpool.tile([128, 4], F32, tag="call")

            def stream_tile(srcs, nch, vals):
                """DMA [128, CH] uint8 chunks + one fused DVE
                multiply-reduce each into vals[:, c].  The one-hot is
                streamed as uint8 (lossless: values are 0/1), quartering
                HBM traffic; the DVE pass is the kernel bottleneck."""
                for c in range(nch):
                    chunk = dpool.tile([128, CH], U8, tag="chunk")
                    for src, p0, p1 in srcs:
                        nc.sync.dma_start(
                            out=chunk[p0:p1, :], in_=src[:, c * CH:(c + 1) * CH]
                        )
                    prod = ppool.tile([128, CH], F32, tag="prod")
                    nc.vector.scalar_tensor_tensor(
                        out=prod[:], in0=chunk[:], scalar=1.0, in1=iota_t[:],
                        op0=OP.mult, op1=OP.mult,
                        accum_out=vals[:, c:c + 1],
                    )

            def extract(vals, wcbits, ncol, cnt_out, off_out, tag):
                """cnt = (row had a 1); off = CH*c_hit + j_hit (0 if none).
                The hit-chunk index c is recovered bit-by-bit: one weighted
                column reduce per bit of c (weight = that bit of each
                column's chunk index), then is_ge(MARK) -> bit, scaled by
                CH*2^b and summed.  No division needed anywhere.  Heavy
                reduces on DVE (no DMA deps); scalar chain on Pool."""
                S1 = spool.tile([128, 1], F32, tag=f"S1{tag}")
                nc.vector.tensor_reduce(out=S1[:], in_=vals[:], axis=AX,
                                        op=OP.add)
                nbits = len(wcbits)
                bbs = spool.tile([128, nbits], F32, tag=f"bbs{tag}")
                for b, wc in enumerate(wcbits):
                    S2 = spool.tile([128, 1], F32, tag=f"S2{tag}{b}")
                    junk = spool.tile([128, ncol], F32, tag=f"jk{ncol}")
                    nc.vector.scalar_tensor_tensor(
                        out=junk[:], in0=vals[:], scalar=1.0, in1=wc,
                        op0=OP.mult, op1=OP.mult, accum_out=S2[:],
                    )
                    nc.vector.tensor_scalar(
                        out=bbs[:, b:b + 1], in0=S2[:], scalar1=MARK,
                        scalar2=None, op0=OP.is_ge,
                    )
                nc.vector.tensor_scalar(
                    out=cnt_out, in0=S1[:], scalar1=MARK, scalar2=None, op0=OP.is_ge
                )
                # acc = sum_b bit_b * CH*2^b in ONE fused weighted reduce
                junkb = spool.tile([128, nbits], F32, tag=f"jb{tag}")
                acc = spool.tile([128, 1], F32, tag=f"acc{tag}")
                nc.vector.scalar_tensor_tensor(
                    out=junkb[:], in0=bbs[:], scalar=1.0,
                    in1=wpow_t[:, :nbits],
                    op0=OP.mult, op1=OP.mult, accum_out=acc[:],
                )
                # off = max(acc - MARK + S1, 0): a no-hit row gives acc=0,
                # S1=0 -> clamps to 0; keeps cnt off this path.  The clamp
                # writes off_out directly (an i32 tile for the gather path).
                aj = spool.tile([128, 1], F32, tag=f"aj{tag}")
                nc.vector.scalar_tensor_tensor(
                    out=aj[:], in0=acc[:], scalar=-MARK, in1=S1[:],
                    op0=OP.add, op1=OP.add,
                )
                nc.vector.tensor_scalar(
                    out=off_out, in0=aj[:], scalar1=0.0, scalar2=None, op0=OP.max
                )

            # ---------------- T0: vo quarters ----------------
            vals0 = spool.tile([128, 2], F32, tag="vals0")
            stream_tile([(vo_q, 0, 128)], 2, vals0)

            # small constants: two merged DMAs on the ACT queue (after the
            # first big chunks, one HWDGE generation each)
            cc_t = cpool.tile([128, 62], F32, tag="cc")
            nc.scalar.dma_start(out=cc_t[:], in_=cc[:])
            reps_all = cpool.tile([32, 4 * 128], F32, tag="reps")
            nc.scalar.dma_start(out=reps_all[:], in_=reps[:])
            qbaseM_t = cc_t[:, 0:1]
            wc2_bits = [cc_t[:, 1:3]]
            wc8_bits = [cc_t[:, 3 + 8 * b:11 + 8 * b] for b in range(3)]
            foldq_t = cc_t[:, 27:59]
            wpow_t = cc_t[:, 59:62]
            reps_t = [reps_all[:, 128 * t:128 * (t + 1)] for t in range(4)]

            cnt0 = spool.tile([128, 1], F32, tag="cnt0")
            off0 = spool.tile([128, 1], F32, tag="off0")
            extract(vals0, wc2_bits, 2, cnt0[:], off0[:], "t0")
            # X = cnt*(MARK + 12500 q) + off ; fold quarters -> [32, 1]
            xq = spool.tile([128, 1], F32, tag="xq")
            nc.vector.tensor_tensor(out=xq[:], in0=cnt0[:], in1=qbaseM_t, op=OP.mult)
            x2 = spool.tile([128, 1], F32, tag="x2")
            nc.vector.tensor_tensor(out=x2[:], in0=xq[:], in1=off0[:], op=OP.add)
            pfold = psum_pool.tile([32, 1], F32, tag="p32")
            nc.tensor.matmul(out=pfold[:], lhsT=foldq_t, rhs=x2[:],
                             start=True, stop=True)
            ofs_v = spool.tile([32, 1], I32, tag="ofsv")
            nc.vector.tensor_scalar(
                out=ofs_v[:], in0=pfold[:], scalar1=MARK, scalar2=None,
                op0=OP.subtract,
            )
            voV = cpool.tile([32, EMB], F32, tag="voV")
            nc.gpsimd.indirect_dma_start(
                out=voV[:], out_offset=None, in_=V[:],
                in_offset=bass.IndirectOffsetOnAxis(ap=ofs_v[:], axis=0),
            )

            # ---------------- T1..T4: vi + neg rows ----------------
            # The per-tile dot runs as Pool multiply + ACT accumulate —
            # neither touches the DVE streaming queue, so a gather that
            # lands late can only block Pool (whose next deadline is a
            # full tile away).
            for t in range(4):
                last = t == 3
                ncol = NCH
                vals = spool.tile([128, ncol], F32, tag=f"vals{ncol}")
                stream_tile(big[t], NCH, vals)
                ofs_u = spool.tile([128, 1], I32, tag="ofsu")
                extract(vals, wc8_bits, ncol,
                        call[:, t:t + 1], ofs_u[:], "tb")
                rowU = gpool.tile([128, EMB], F32, tag="rowU")
                nc.gpsimd.indirect_dma_start(
                    out=rowU[:], out_offset=None, in_=U[:],
                    in_offset=bass.IndirectOffsetOnAxis(ap=ofs_u[:], axis=0),
                )
                pB = psum_pool.tile([128, EMB], F32, tag="pB")
                nc.tensor.matmul(out=pB[:], lhsT=reps_t[t], rhs=voV[:],
                                 start=True, stop=True)
                voB = gpool.tile([128, EMB], F32, tag="voB")
                nc.scalar.activation(out=voB[:], in_=pB[:], func=ACTF.Copy)
                prodB = gpool.tile([128, EMB], F32, tag="prodB")
                if last:
                    # tail: one fused DVE op (DVE is idle by now)
                    nc.vector.scalar_tensor_tensor(
                        out=prodB[:], in0=rowU[:], scalar=1.0, in1=voB[:],
                        op0=OP.mult, op1=OP.mult, accum_out=dall[:, t:t + 1],
                    )
                else:
                    # mid-stream: keep the gather-dependent dot off DVE
                    nc.gpsimd.tensor_tensor(
                        out=prodB[:], in0=rowU[:], in1=voB[:], op=OP.mult
                    )
                    nc.scalar.activation(
                        out=prodB[:], in_=prodB[:], func=ACTF.Copy,
                        accum_out=dall[:, t:t + 1],
                    )

            # c_out rides the idle SP queue (off the critical tail); d_out
            # on ACT (one DVE->ACT hop after the fused T4 dot)
            nc.sync.dma_start(out=c_out[:], in_=call[:])
            nc.scalar.dma_start(out=d_out[:], in_=dall[:])

    _split_multi_waits(nc)
    mybir.codegen_inst_isa_subclasses(nc)
    return nc


def _consts():
    p = np.arange(128)
    qbaseM = (MARK + (p % 4) * QW).astype(np.float32).reshape(128, 1)
    wc2 = np.tile(np.arange(2, dtype=np.float32), (128, 1))
    wc8 = np.tile(np.arange(8, dtype=np.float32), (128, 1))
    c8 = np.arange(8)
    wc8b = np.concatenate(
        [np.tile(((c8 >> b) & 1).astype(np.float32), (128, 1)) for b in range(3)],
        axis=1)
    foldq = np.zeros((128, 32), np.float32)
    foldq[p, p // 4] = 1.0
    wpow = np.tile(np.array([6250.0, 12500.0, 25000.0],
                            dtype=np.float32), (128, 1))
    cc = np.concatenate([qbaseM, wc2, wc8b, foldq, wpow], axis=1)
    # reps[t, b, p] = 1 iff partition p of tile t holds a row of batch b
    bmap = np.empty((4, 128), np.int64)
    bmap[0] = p // CTX                                   # vi rows 0..127
    bmap[1, :64] = (128 + p[:64]) // CTX                 # vi rows 128..191
    bmap[1, 64:] = (p[64:] - 64) // K                    # ng rows 0..63
    bmap[2] = (64 + p) // K                              # ng rows 64..191
    bmap[3] = (192 + p) // K                             # ng rows 192..319
    reps = np.zeros((4, 32, 128), np.float32)
    for t in range(4):
        reps[t, bmap[t], p] = 1.0
    reps = reps.transpose(1, 0, 2).reshape(32, 4 * 128)
    return cc, reps


_CACHE = {}


def kernel(vo, vi, neg_samples, V, U):
    if "nc" not in _CACHE:
        _CACHE["nc"] = _build()
        _CACHE["consts"] = _consts()
    nc = _CACHE["nc"]
    cc, reps = _CACHE["consts"]

    # one-hot payloads are exactly {0.0, 1.0}: recode to uint8 so the
    # device streams 4x fewer bytes (lossless)
    vo = np.ascontiguousarray(vo, dtype=np.float32).astype(np.uint8)
    vi = np.ascontiguousarray(vi, dtype=np.float32).astype(np.uint8)
    neg = np.ascontiguousarray(neg_samples, dtype=np.float32).astype(np.uint8)
    V = np.ascontiguousarray(V, dtype=np.float32)
    U = np.ascontiguousarray(U, dtype=np.float32)

    in_maps = []
    for c in range(NCORES):
        sl = slice(c * BPC, (c + 1) * BPC)
        in_maps.append({
            "vo": vo[sl],
            "vi": vi[sl].reshape(NV, VOC),
            "ng": neg[sl].reshape(NN, VOC),
            "V": V,
            "U": U,
            "cc": cc, "reps": reps,
        })

    res = run_bass_kernel_spmd(nc, in_maps, list(range(NCORES)))
    obs = []
    for r in res.results:
        d = r["dout"]                                  # [128, 4]
        cc = r["cout"]                                 # [128, 4]
        d_vi = np.concatenate([d[:, 0], d[:64, 1]]).reshape(BPC, CTX)
        c_vi = np.concatenate([cc[:, 0], cc[:64, 1]]).reshape(BPC, CTX)
        d_ng = np.concatenate([d[64:, 1], d[:, 2], d[:, 3]]).reshape(BPC, K)
        lp = (d_vi * c_vi).sum(axis=1)
        ms = c_vi.sum(axis=1)
        x = lp / ms
        left = -np.log1p(np.exp(-x))
        right = (-np.log1p(np.exp(d_ng))).sum(axis=1)
        obs.append(-(left + right))
    ob = np.concatenate(obs)
    return np.float32(ob.mean(dtype=np.float64))
